# revision 49
# baseline (speedup 1.0000x reference)
"""ExemplarAttention Trainium2 kernel (8 NeuronCores, exemplar-sharded).

logits[b,c] = gamma * log(sum_{n:label[n]=c} exp(-beta * sum_k w_k (x[b,k]-e[n,k])^2) + eps)

Strategy (transposed layout, exemplars on partitions):
  - Shard the N=16384 exemplar bank across 8 cores (2048 each, 16 tiles of
    128); replicate the batch side. Per n-tile the PE computes
    psum[n, m] = S * cross[n, m] with e-features as the stationary operand
    (fp8 DoubleRow, 2 groups of K=256) and S*x*w as the moving operand
    (m = full B = 1024 free columns, so LDWEIGHTS amortizes).
  - ScalarE: sim[n, m] = exp((2*beta/S)*psum + (-beta*e2w)[n]) -- e2w is a
    per-partition bias, so no PSUM-prefill matmuls are needed at all. The
    exp(-beta*x2w[m]) factor comes out of the log and is applied on host.
  - Class scatter-add runs on the PE as a one-hot matmul: sim is written in
    fp8 and consumed as a DoubleRow moving operand, so each K=256 pass sums
    two n-tiles: cls_psum[c, m] += onehot[n, c].T @ sim[n, m]. Half a pass
    per column instead of the full pass a PSUM-prefill (or a 1x-rate DVE
    segment-reduce) would cost.
  - Each core DMAs out its partial class sums; the host sums the partials
    (the unshard step for an associative segment-sum), multiplies by
    exp(-beta*x2w), and applies gamma*log(.+eps) in float64.
  - DMA: per-transfer serialized fixed cost (~1.5us HWDGE FIFO + HBM
    receipt) dominates over bandwidth, so inputs ride in 6 contiguous
    transfers ordered first-needed-first across the two rings.
"""

import os
from contextlib import ExitStack

import numpy as np

B, N, D, C = 1024, 16384, 512, 10
NCORES = 8
N_LOC = N // NCORES          # 2048 exemplars per core
NTILES = N_LOC // 128        # 16 n-tiles of 128 exemplars
NPAIR = NTILES // 2          # 8 DoubleRow pairs for the one-hot matmul
NG = 2                       # DoubleRow groups over D=512 (K=256 each)
MT = 512                     # matmul free-dim chunk (1 psum bank)
CP = 16                      # one-hot columns padded 10 -> 16 (DR step%16)
EPS = 1e-9
S_SCALE = 128.0              # fp8 scale applied to x*w
# e_t regions (in n-tiles): head gates the first cross matmuls, mid/tail
# stream behind the critical transfers.
REGIONS = [(0, 4), (4, 6), (10, 6)]
# PE warmup matmuls (no DMA deps): keep the PE busy from the end of the
# preamble until xw/e_head arrive so the HAM clock gate opens early.
N_WARMUP_MM = 8

_prog_cache = {}


def _np_dt(mybir, name):
    return mybir.dt.np(getattr(mybir.dt, name))


def _build_program(beta):
    import concourse.bass as bass  # noqa: F401
    import concourse.tile as tile
    from concourse import bacc, mybir

    fp8 = mybir.dt.float8e4
    f32 = mybir.dt.float32

    nc = bacc.Bacc("TRN2", target_bir_lowering=False, debug=False,
                   num_devices=NCORES)

    # e region r-major: e_reg[r, g, s, n] = e[n_glob, (2g+s)*128 + r]
    e_d = [nc.dram_tensor(f"e_r{i}", [128, NG, 2, 128 * w], fp8,
                          kind="ExternalInput").ap()
           for i, (t0, w) in enumerate(REGIONS)]
    # xw_t[g][r, s, m] = S * x[m, (2g+s)*128 + r] * w[...]  (moving operand;
    # one tensor per group so the two rings can carry them in parallel)
    xw_d = [nc.dram_tensor(f"xw_g{g}", [128, 2, B], fp8,
                           kind="ExternalInput").ap() for g in range(NG)]
    # oh[r, s, p*CP + c] = 1 if label[(2p+s)*128 + r] == c
    oh_d = nc.dram_tensor("oh", [128, 2, NPAIR * CP], fp8,
                          kind="ExternalInput").ap()
    # bias[r, t] = -beta * e2w[t*128 + r]
    bias_d = nc.dram_tensor("bias", [128, NTILES], f32,
                            kind="ExternalInput").ap()
    bf16 = mybir.dt.bfloat16
    out_d = nc.dram_tensor("cls", [CP, B], bf16, kind="ExternalOutput").ap()

    act_scale = float(2.0 * beta / S_SCALE)
    DR = mybir.MatmulPerfMode.DoubleRow

    with tile.TileContext(nc) as tc, ExitStack() as ctx:
        singles = ctx.enter_context(tc.tile_pool(name="singles", bufs=1))
        sim_pool = ctx.enter_context(tc.tile_pool(name="sim", bufs=4))
        psum_pool = ctx.enter_context(tc.tile_pool(name="ps", bufs=3,
                                                   space="PSUM"))
        cls_pool = ctx.enter_context(tc.tile_pool(name="cls", bufs=1,
                                                  space="PSUM"))

        # Dummy activation so the ACT exp-table load happens during the DMA
        # startup window instead of blocking the first real exp.
        dummy = singles.tile([128, 1], f32)
        nc.vector.memset(dummy[:, :], 0.0)
        nc.scalar.activation(out=dummy[:, :], in_=dummy[:, :],
                             func=mybir.ActivationFunctionType.Exp, scale=1.0)

        # Warmup matmuls from a memset tile: full-array PE activity right
        # after the preamble opens the HAM clock gate while the DMAs run.
        # The memset rides GpSimd (free ~1.2us before DVE) so the PE starts
        # that much earlier.
        dmy = singles.tile([128, 128 + MT], mybir.dt.bfloat16)
        nc.gpsimd.memset(dmy[:, :], 0.0)
        ps_w = psum_pool.tile([128, B], f32, tag="ps", name="ps_w")
        for _ in range(N_WARMUP_MM):
            nc.tensor.matmul(ps_w[:, 0:MT], lhsT=dmy[:, 0:128],
                             rhs=dmy[:, 128:], start=True, stop=True)

        # Input transfers, first-needed-first on each ring. xw group 0 rides
        # scalar while e-head + xw group 1 ride sync, so the g0 matmuls of
        # the first tiles can start before g1's moving operand lands.
        e_sb = [singles.tile([128, NG, 2, 128 * w], fp8, name=f"er{i}")
                for i, (t0, w) in enumerate(REGIONS)]
        xw_sb = [singles.tile([128, 2, B], fp8, name=f"xw{g}")
                 for g in range(NG)]
        bias_sb = singles.tile([128, NTILES], f32)
        oh_sb = singles.tile([128, 2, NPAIR * CP], fp8)

        # Critical window: each ring carries ONLY first-needed bytes (the
        # SDMA engines round-robin across rings at packet granularity, so a
        # bulk transfer on any ring steals wire from every ring's critical
        # load). Bulk e-tails queue BEHIND xw g0 on the scalar ring --
        # per-ring FIFO defers them until the critical set has drained.
        nc.sync.dma_start(out=e_sb[0][:, :, :, :], in_=e_d[0][:, :, :, :])
        nc.scalar.dma_start(out=xw_sb[0][:, :, :], in_=xw_d[0][:, :, :])
        nc.gpsimd.dma_start(out=xw_sb[1][:, :, :], in_=xw_d[1][:, :, :])
        nc.sync.dma_start(out=bias_sb[:, :], in_=bias_d[:, :])
        nc.sync.dma_start(out=oh_sb[:, :, :], in_=oh_d[:, :, :])
        nc.scalar.dma_start(out=e_sb[1][:, :, :, :], in_=e_d[1][:, :, :, :])
        nc.scalar.dma_start(out=e_sb[2][:, :, :, :], in_=e_d[2][:, :, :, :])

        def et_lhsT(g, t):
            for i, (t0, w) in enumerate(REGIONS):
                if t0 <= t < t0 + w:
                    off = 128 * (t - t0)
                    return e_sb[i][:, g, :, off:off + 128]

        # One class accumulator for all 8 pairs (a recycled accumulator
        # tripped a missing write-after-read dependency -- Tile did not
        # order the next accumulation group after the copy-out).
        cls_ps = cls_pool.tile([128, B], f32, name="cls_ps")

        def emit_oh(p, sim_sb):
            for j in range(B // MT):
                cs = slice(j * MT, (j + 1) * MT)
                nc.tensor.matmul(
                    cls_ps[0:CP, cs],
                    lhsT=oh_sb[:, :, p * CP:(p + 1) * CP],
                    rhs=sim_sb[:, :, cs],
                    start=(p == 0), stop=(p == NPAIR - 1),
                    perf_mode=DR)

        def emit_out():
            # Copy stays on DVE (idle; ScalarE is the bottleneck engine and
            # dispatches late at the tail). bf16 runs the PSUM read at 2x
            # and halves the out-DMA; the partial sums only lose ~0.2%.
            cls_out = singles.tile([CP, B], bf16, name="co")
            nc.vector.tensor_copy(cls_out[:, :], cls_ps[0:CP, :])
            nc.sync.dma_start(out=out_d[:, :], in_=cls_out[:, :])

        def emit_cross(ps, g, t):
            lhsT = et_lhsT(g, t)
            for j in range(B // MT):
                cs = slice(j * MT, (j + 1) * MT)
                nc.tensor.matmul(
                    ps[:, cs], lhsT=lhsT,
                    rhs=xw_sb[g][:, :, cs],
                    start=(g == 0), stop=(g == NG - 1),
                    perf_mode=DR)

        def emit_exp(ps, sim_sb, s, t):
            # The last tile's exp runs in two halves so the final one-hot
            # matmul (and the tail copy/DMA chain behind it) starts half an
            # instruction earlier.
            halves = 2 if t == NTILES - 1 else 1
            hw = B // halves
            for hh in range(halves):
                nc.scalar.activation(
                    out=sim_sb[:, s, hh * hw:(hh + 1) * hw],
                    in_=ps[:, hh * hw:(hh + 1) * hw],
                    func=mybir.ActivationFunctionType.Exp,
                    bias=bias_sb[:, t:t + 1],
                    scale=act_scale,
                )

        # One-hot class-sum matmuls trail the cross matmuls by one pair so
        # the PE never waits on ScalarE's exp (which lags ~1us behind).
        sim_tiles = []
        for p in range(NPAIR):
            sim_sb = sim_pool.tile([128, 2, B], fp8, tag="sim")
            sim_tiles.append(sim_sb)
            for s in range(2):
                t = 2 * p + s
                ps = psum_pool.tile([128, B], f32, tag="ps", name=f"ps{t}")
                if t == 0:
                    # Normal LDW order (g0 then g1), but exp the first
                    # m-half right after g1's first chunk completes it --
                    # the saturated ScalarE stream starts one MM earlier.
                    emit_cross(ps, 0, t)
                    lhsT = et_lhsT(1, t)
                    for j in range(B // MT):
                        cs = slice(j * MT, (j + 1) * MT)
                        nc.tensor.matmul(
                            ps[:, cs], lhsT=lhsT, rhs=xw_sb[1][:, :, cs],
                            start=False, stop=True, perf_mode=DR)
                        nc.scalar.activation(
                            out=sim_sb[:, s, cs], in_=ps[:, cs],
                            func=mybir.ActivationFunctionType.Exp,
                            bias=bias_sb[:, t:t + 1], scale=act_scale)
                    continue
                emit_cross(ps, 0, t)
                emit_cross(ps, 1, t)
                emit_exp(ps, sim_sb, s, t)
            if p >= 1:
                emit_oh(p - 1, sim_tiles[p - 1])
        emit_oh(NPAIR - 1, sim_tiles[NPAIR - 1])
        emit_out()

    nc.compile()
    return nc


def _prepare(x, ex_feats, ex_labels, w_unconstrained, gamma_unconstrained,
             beta_unconstrained):
    from concourse import mybir

    x = np.asarray(x, dtype=np.float64)
    e = np.asarray(ex_feats, dtype=np.float64)
    labels = np.asarray(ex_labels).astype(np.int64)
    wu = np.asarray(w_unconstrained, dtype=np.float64)

    beta = float(np.log1p(np.exp(np.float64(beta_unconstrained)))) + EPS
    gamma = float(np.log1p(np.exp(np.float64(gamma_unconstrained)))) + EPS
    wexp = np.exp(wu - wu.max())
    w = wexp / wexp.sum() + EPS

    fp8 = _np_dt(mybir, "float8e4")

    xw = (S_SCALE * x * w[None, :])                   # (B, D)
    x2w = (x * x) @ w                                 # (B,)
    e2w = (e * e) @ w                                 # (N,)

    # xw_t[r, g, s, m] = S*xw[m, (2g+s)*128 + r]; shipped per group
    xw_t = np.ascontiguousarray(
        xw.T.reshape(NG, 2, 128, B).transpose(2, 0, 1, 3)).astype(fp8)
    xw_g = [np.ascontiguousarray(xw_t[:, g]) for g in range(NG)]

    per_core = []
    for cid in range(NCORES):
        sl = slice(cid * N_LOC, (cid + 1) * N_LOC)
        e_c = e[sl]                                   # (N_LOC, D)
        # e_r[r, g, s, n] = e_c[n, (2g+s)*128 + r]
        e_r = np.ascontiguousarray(
            e_c.T.reshape(NG, 2, 128, N_LOC).transpose(2, 0, 1, 3)).astype(fp8)
        core_in = {
            "xw_g0": xw_g[0],
            "xw_g1": xw_g[1],
            "bias": np.ascontiguousarray(
                (-beta * e2w[sl]).astype(np.float32).reshape(NTILES, 128).T),
        }
        for i, (t0, wdt) in enumerate(REGIONS):
            core_in[f"e_r{i}"] = np.ascontiguousarray(
                e_r[:, :, :, 128 * t0:128 * (t0 + wdt)])
        lab_c = labels[sl].reshape(NTILES, 128)       # (t, r)
        oh = np.zeros((128, 2, NPAIR * CP), dtype=np.float32)
        for p in range(NPAIR):
            for s in range(2):
                lab = lab_c[2 * p + s]                # (128,)
                oh[np.arange(128), s, p * CP + lab] = 1.0
        core_in["oh"] = oh.astype(fp8)
        per_core.append(core_in)
    return per_core, beta, gamma, x2w


def kernel(x, ex_feats, ex_labels, w_unconstrained, gamma_unconstrained,
           beta_unconstrained, _want_results=False, **run_kwargs):
    from concourse.bass_utils import run_bass_kernel_spmd

    per_core, beta, gamma, x2w = _prepare(
        x, ex_feats, ex_labels, w_unconstrained, gamma_unconstrained,
        beta_unconstrained)

    key = round(beta, 12)
    if key not in _prog_cache:
        _prog_cache[key] = _build_program(beta)
    nc = _prog_cache[key]

    res = run_bass_kernel_spmd(nc, per_core, list(range(NCORES)), **run_kwargs)
    # Unshard: the per-class partial sums are associative -- sum the 16
    # partials, then apply the factored-out exp(-beta*x2w) and gamma*log.
    parts = np.zeros((CP, B), dtype=np.float64)
    for cid in range(NCORES):
        parts += np.asarray(res.results[cid]["cls"], dtype=np.float64)
    class_sum = parts[:C, :].T * np.exp(-beta * x2w)[:, None]   # (B, C)
    out = (gamma * np.log(class_sum + EPS)).astype(np.float32)
    if _want_results:
        return out, res
    return out


# revision 50
# speedup vs baseline: 1.0240x; 1.0240x over previous
"""ExemplarAttention Trainium2 kernel (8 NeuronCores, exemplar-sharded).

logits[b,c] = gamma * log(sum_{n:label[n]=c} exp(-beta * sum_k w_k (x[b,k]-e[n,k])^2) + eps)

Strategy (transposed layout, exemplars on partitions):
  - Shard the N=16384 exemplar bank across 8 cores (2048 each, 16 tiles of
    128); replicate the batch side. Per n-tile the PE computes
    psum[n, m] = S * cross[n, m] with e-features as the stationary operand
    (fp8 DoubleRow, 2 groups of K=256) and S*x*w as the moving operand
    (m = full B = 1024 free columns, so LDWEIGHTS amortizes).
  - ScalarE: sim[n, m] = exp((2*beta/S)*psum + (-beta*e2w)[n]) -- e2w is a
    per-partition bias, so no PSUM-prefill matmuls are needed at all. The
    exp(-beta*x2w[m]) factor comes out of the log and is applied on host.
  - Class scatter-add runs on the PE as a one-hot matmul: sim is written in
    fp8 and consumed as a DoubleRow moving operand, so each K=256 pass sums
    two n-tiles: cls_psum[c, m] += onehot[n, c].T @ sim[n, m]. Half a pass
    per column instead of the full pass a PSUM-prefill (or a 1x-rate DVE
    segment-reduce) would cost.
  - Each core DMAs out its partial class sums; the host sums the partials
    (the unshard step for an associative segment-sum), multiplies by
    exp(-beta*x2w), and applies gamma*log(.+eps) in float64.
  - DMA: per-transfer serialized fixed cost (~1.5us HWDGE FIFO + HBM
    receipt) dominates over bandwidth, so inputs ride in 6 contiguous
    transfers ordered first-needed-first across the two rings.
"""

import os
from contextlib import ExitStack

import numpy as np

B, N, D, C = 1024, 16384, 512, 10
NCORES = 8
N_LOC = N // NCORES          # 2048 exemplars per core
NTILES = N_LOC // 128        # 16 n-tiles of 128 exemplars
NPAIR = NTILES // 2          # 8 DoubleRow pairs for the one-hot matmul
NG = 2                       # DoubleRow groups over D=512 (K=256 each)
MT = 512                     # matmul free-dim chunk (1 psum bank)
CP = 16                      # one-hot columns padded 10 -> 16 (DR step%16)
EPS = 1e-9
S_SCALE = 128.0              # fp8 scale applied to x*w
# e_t regions (in n-tiles): head gates the first cross matmuls, mid/tail
# stream behind the critical transfers.
REGIONS = [(0, 4), (4, 6), (10, 6)]
# PE warmup matmuls (no DMA deps): keep the PE busy from the end of the
# preamble until xw/e_head arrive so the HAM clock gate opens early.
N_WARMUP_MM = 8

_prog_cache = {}


def _np_dt(mybir, name):
    return mybir.dt.np(getattr(mybir.dt, name))


def _build_program(beta):
    import concourse.bass as bass  # noqa: F401
    import concourse.tile as tile
    from concourse import bacc, mybir

    fp8 = mybir.dt.float8e4
    f32 = mybir.dt.float32

    nc = bacc.Bacc("TRN2", target_bir_lowering=False, debug=False,
                   num_devices=NCORES)

    # e region r-major: e_reg[r, g, s, n] = e[n_glob, (2g+s)*128 + r]
    e_d = [nc.dram_tensor(f"e_r{i}", [128, NG, 2, 128 * w], fp8,
                          kind="ExternalInput").ap()
           for i, (t0, w) in enumerate(REGIONS)]
    # xw_t[g][r, s, m] = S * x[m, (2g+s)*128 + r] * w[...]  (moving operand;
    # one tensor per group so the two rings can carry them in parallel)
    xw_d = [nc.dram_tensor(f"xw_g{g}", [128, 2, B], fp8,
                           kind="ExternalInput").ap() for g in range(NG)]
    # oh[r, s, p*CP + c] = 1 if label[(2p+s)*128 + r] == c
    oh_d = nc.dram_tensor("oh", [128, 2, NPAIR * CP], fp8,
                          kind="ExternalInput").ap()
    # bias[r, t] = -beta * e2w[t*128 + r]
    bias_d = nc.dram_tensor("bias", [128, NTILES], f32,
                            kind="ExternalInput").ap()
    bf16 = mybir.dt.bfloat16
    out_d = nc.dram_tensor("cls", [CP, B], bf16, kind="ExternalOutput").ap()

    act_scale = float(2.0 * beta / S_SCALE)
    DR = mybir.MatmulPerfMode.DoubleRow

    with tile.TileContext(nc) as tc, ExitStack() as ctx:
        singles = ctx.enter_context(tc.tile_pool(name="singles", bufs=1))
        sim_pool = ctx.enter_context(tc.tile_pool(name="sim", bufs=3))
        psum_pool = ctx.enter_context(tc.tile_pool(name="ps", bufs=3,
                                                   space="PSUM"))
        cls_pool = ctx.enter_context(tc.tile_pool(name="cls", bufs=1,
                                                  space="PSUM"))

        # Dummy activation so the ACT exp-table load happens during the DMA
        # startup window instead of blocking the first real exp.
        dummy = singles.tile([128, 1], f32)
        nc.vector.memset(dummy[:, :], 0.0)
        nc.scalar.activation(out=dummy[:, :], in_=dummy[:, :],
                             func=mybir.ActivationFunctionType.Exp, scale=1.0)

        # Warmup matmuls from a memset tile: full-array PE activity right
        # after the preamble opens the HAM clock gate while the DMAs run.
        # The memset rides GpSimd (free ~1.2us before DVE) so the PE starts
        # that much earlier.
        dmy = singles.tile([128, 128 + MT], mybir.dt.bfloat16)
        nc.gpsimd.memset(dmy[:, :], 0.0)
        ps_w = psum_pool.tile([128, B], f32, tag="ps", name="ps_w")
        for _ in range(N_WARMUP_MM):
            nc.tensor.matmul(ps_w[:, 0:MT], lhsT=dmy[:, 0:128],
                             rhs=dmy[:, 128:], start=True, stop=True)

        # Input transfers, first-needed-first on each ring. xw group 0 rides
        # scalar while e-head + xw group 1 ride sync, so the g0 matmuls of
        # the first tiles can start before g1's moving operand lands.
        e_sb = [singles.tile([128, NG, 2, 128 * w], fp8, name=f"er{i}")
                for i, (t0, w) in enumerate(REGIONS)]
        xw_sb = [singles.tile([128, 2, B], fp8, name=f"xw{g}")
                 for g in range(NG)]
        bias_sb = singles.tile([128, NTILES], f32)
        oh_sb = singles.tile([128, 2, NPAIR * CP], fp8)

        # Critical window: each ring carries ONLY first-needed bytes (the
        # SDMA engines round-robin across rings at packet granularity, so a
        # bulk transfer on any ring steals wire from every ring's critical
        # load). Bulk e-tails queue BEHIND xw g0 on the scalar ring --
        # per-ring FIFO defers them until the critical set has drained.
        nc.sync.dma_start(out=e_sb[0][:, :, :, :], in_=e_d[0][:, :, :, :])
        nc.scalar.dma_start(out=xw_sb[0][:, :, :], in_=xw_d[0][:, :, :])
        nc.gpsimd.dma_start(out=xw_sb[1][:, :, :], in_=xw_d[1][:, :, :])
        nc.sync.dma_start(out=bias_sb[:, :], in_=bias_d[:, :])
        nc.sync.dma_start(out=oh_sb[:, :, :], in_=oh_d[:, :, :])
        nc.scalar.dma_start(out=e_sb[1][:, :, :, :], in_=e_d[1][:, :, :, :])
        nc.scalar.dma_start(out=e_sb[2][:, :, :, :], in_=e_d[2][:, :, :, :])

        def et_lhsT(g, t):
            for i, (t0, w) in enumerate(REGIONS):
                if t0 <= t < t0 + w:
                    off = 128 * (t - t0)
                    return e_sb[i][:, g, :, off:off + 128]

        # One class accumulator for all 8 pairs (a recycled accumulator
        # tripped a missing write-after-read dependency -- Tile did not
        # order the next accumulation group after the copy-out).
        cls_ps = cls_pool.tile([128, B], f32, name="cls_ps")

        def emit_oh(p, sim_sb):
            for j in range(B // MT):
                cs = slice(j * MT, (j + 1) * MT)
                nc.tensor.matmul(
                    cls_ps[0:CP, cs],
                    lhsT=oh_sb[:, :, p * CP:(p + 1) * CP],
                    rhs=sim_sb[:, :, cs],
                    start=(p == 0), stop=(p == NPAIR - 1),
                    perf_mode=DR)

        def emit_out():
            # Copy stays on DVE (idle; ScalarE is the bottleneck engine and
            # dispatches late at the tail). bf16 runs the PSUM read at 2x
            # and halves the out-DMA; the partial sums only lose ~0.2%.
            cls_out = singles.tile([CP, B], bf16, name="co")
            nc.vector.tensor_copy(cls_out[:, :], cls_ps[0:CP, :])
            nc.sync.dma_start(out=out_d[:, :], in_=cls_out[:, :])

        def emit_cross(ps, g, t):
            lhsT = et_lhsT(g, t)
            for j in range(B // MT):
                cs = slice(j * MT, (j + 1) * MT)
                nc.tensor.matmul(
                    ps[:, cs], lhsT=lhsT,
                    rhs=xw_sb[g][:, :, cs],
                    start=(g == 0), stop=(g == NG - 1),
                    perf_mode=DR)

        def emit_exp(ps, sim_sb, s, t):
            # The last tile's exp runs in two halves so the final one-hot
            # matmul (and the tail copy/DMA chain behind it) starts half an
            # instruction earlier.
            halves = 2 if t == NTILES - 1 else 1
            hw = B // halves
            for hh in range(halves):
                nc.scalar.activation(
                    out=sim_sb[:, s, hh * hw:(hh + 1) * hw],
                    in_=ps[:, hh * hw:(hh + 1) * hw],
                    func=mybir.ActivationFunctionType.Exp,
                    bias=bias_sb[:, t:t + 1],
                    scale=act_scale,
                )

        # One-hot class-sum matmuls trail the cross matmuls by one pair so
        # the PE never waits on ScalarE's exp (which lags ~1us behind).
        sim_tiles = []
        for p in range(NPAIR):
            sim_sb = sim_pool.tile([128, 2, B], fp8, tag="sim")
            sim_tiles.append(sim_sb)
            for s in range(2):
                t = 2 * p + s
                ps = psum_pool.tile([128, B], f32, tag="ps", name=f"ps{t}")
                emit_cross(ps, 0, t)
                emit_cross(ps, 1, t)
                emit_exp(ps, sim_sb, s, t)
            if p >= 1:
                emit_oh(p - 1, sim_tiles[p - 1])
        emit_oh(NPAIR - 1, sim_tiles[NPAIR - 1])
        emit_out()

    nc.compile()
    return nc


def _prepare(x, ex_feats, ex_labels, w_unconstrained, gamma_unconstrained,
             beta_unconstrained):
    from concourse import mybir

    x = np.asarray(x, dtype=np.float64)
    e = np.asarray(ex_feats, dtype=np.float64)
    labels = np.asarray(ex_labels).astype(np.int64)
    wu = np.asarray(w_unconstrained, dtype=np.float64)

    beta = float(np.log1p(np.exp(np.float64(beta_unconstrained)))) + EPS
    gamma = float(np.log1p(np.exp(np.float64(gamma_unconstrained)))) + EPS
    wexp = np.exp(wu - wu.max())
    w = wexp / wexp.sum() + EPS

    fp8 = _np_dt(mybir, "float8e4")

    xw = (S_SCALE * x * w[None, :])                   # (B, D)
    x2w = (x * x) @ w                                 # (B,)
    e2w = (e * e) @ w                                 # (N,)

    # xw_t[r, g, s, m] = S*xw[m, (2g+s)*128 + r]; shipped per group
    xw_t = np.ascontiguousarray(
        xw.T.reshape(NG, 2, 128, B).transpose(2, 0, 1, 3)).astype(fp8)
    xw_g = [np.ascontiguousarray(xw_t[:, g]) for g in range(NG)]

    per_core = []
    for cid in range(NCORES):
        sl = slice(cid * N_LOC, (cid + 1) * N_LOC)
        e_c = e[sl]                                   # (N_LOC, D)
        # e_r[r, g, s, n] = e_c[n, (2g+s)*128 + r]
        e_r = np.ascontiguousarray(
            e_c.T.reshape(NG, 2, 128, N_LOC).transpose(2, 0, 1, 3)).astype(fp8)
        core_in = {
            "xw_g0": xw_g[0],
            "xw_g1": xw_g[1],
            "bias": np.ascontiguousarray(
                (-beta * e2w[sl]).astype(np.float32).reshape(NTILES, 128).T),
        }
        for i, (t0, wdt) in enumerate(REGIONS):
            core_in[f"e_r{i}"] = np.ascontiguousarray(
                e_r[:, :, :, 128 * t0:128 * (t0 + wdt)])
        lab_c = labels[sl].reshape(NTILES, 128)       # (t, r)
        oh = np.zeros((128, 2, NPAIR * CP), dtype=np.float32)
        for p in range(NPAIR):
            for s in range(2):
                lab = lab_c[2 * p + s]                # (128,)
                oh[np.arange(128), s, p * CP + lab] = 1.0
        core_in["oh"] = oh.astype(fp8)
        per_core.append(core_in)
    return per_core, beta, gamma, x2w


def kernel(x, ex_feats, ex_labels, w_unconstrained, gamma_unconstrained,
           beta_unconstrained, _want_results=False, **run_kwargs):
    from concourse.bass_utils import run_bass_kernel_spmd

    per_core, beta, gamma, x2w = _prepare(
        x, ex_feats, ex_labels, w_unconstrained, gamma_unconstrained,
        beta_unconstrained)

    key = round(beta, 12)
    if key not in _prog_cache:
        _prog_cache[key] = _build_program(beta)
    nc = _prog_cache[key]

    res = run_bass_kernel_spmd(nc, per_core, list(range(NCORES)), **run_kwargs)
    # Unshard: the per-class partial sums are associative -- sum the 16
    # partials, then apply the factored-out exp(-beta*x2w) and gamma*log.
    parts = np.zeros((CP, B), dtype=np.float64)
    for cid in range(NCORES):
        parts += np.asarray(res.results[cid]["cls"], dtype=np.float64)
    class_sum = parts[:C, :].T * np.exp(-beta * x2w)[:, None]   # (B, C)
    out = (gamma * np.log(class_sum + EPS)).astype(np.float32)
    if _want_results:
        return out, res
    return out
